# revision 12
# baseline (speedup 1.0000x reference)
"""MoE (8 experts, top-2, shared expert) Trainium2 kernel.

Strategy (expert-parallel, per sharding hint):
  - Host computes routing (sigmoid gate -> top-2 -> stable sort by expert),
    exactly mirroring the jax reference in fp32 numpy, and plays the role of
    the all-to-all: expert e's tokens (scaled by router scores, bf16,
    zero-padded to capacity C = max expert count) go to core e. The shared
    expert is data-parallel: core i gets tokens [i*256, (i+1)*256).
  - Device does the 4 GEMMs in feature-major layout (tokens on the moving
    free dim) so no on-chip transposes are needed:
        hT  = wu.T.T @ xrT  (bf16, fp32 accum)   -> relu^2 in bf16
        yrT = wd.T.T @ hT   (bf16, fp32 accum)   -> bf16 out
        gT  = su.T.T @ xsT  (fp16)               -> relu^2
        ysT = sd.T.T @ gT   (fp16)               -> fp32 out
    fp16 (10-bit mantissa) gives ~tf32 precision at half the f32 DMA bytes.
  - Host scatters per-expert outputs back to token order, sums top-2 + shared.

Schedule notes (what made it fast):
  - Weights are streamed M-MAJOR (one DRAM param per tensor, tiled
    [128, m, k, 128]) and DMA triggers are issued in exactly the PE's
    consumption order, so the first matmul only waits for xs + su[m0]
    (~0.8 MB) instead of a whole k-major pack.  HWDGE queues are FIFO per
    engine, so trigger order == delivery order with no dep chaining.
  - Two HWDGE engines: shared-expert streams + sd + ys-stores ride the sync
    queues, routed streams + wd + yr-stores ride the scalar (ACT) queues.
  - Token chunks split C into two near-halves (>=233 cols each) so every
    LDWEIGHTS (~97ns) hides under the previous matmul's column stream; the
    baseline's (512, 64) split exposed ~90 cycles per narrow chunk.
  - Up-phase interleaves shared/routed m-tiles (shared first: its input is
    small and lands first); the last up-tile is shared so the down-phase's
    first routed tile never waits on an epilogue.
  - PSUM tile slots are bank-padded (2KB/partition) by the framework, so 6
    in-flight accumulators (2x274 + 2x273 + 2x256 fp32) fit the 8 banks.

Self-contained: hardcodes shapes from the problem spec.
"""

import numpy as np
import ml_dtypes
from contextlib import ExitStack

T = 2048          # tokens (BS*SLEN)
DIM = 1024
E = 8             # experts == cores
TOPK = 2
HID = 1408
NCORES = 8
S = T // NCORES   # shared-expert tokens per core

KD = DIM // 128   # 8  k-tiles contracting over DIM
MH = HID // 128   # 11 m-tiles over hidden
MD = DIM // 128   # 8  m-tiles over model dim

TRACE = False
TRACE_CORES = None
TRACE_DIR = None
LAST_RESULT = None   # BassKernelResults of the last run (for test harness)

_PROG_CACHE = {}

bf16 = ml_dtypes.bfloat16


def _chunks(total):
    """Split total into near-equal chunks of <=512 (PSUM bank width).
    Near-equal (not 512+rest) so every chunk is wide enough to hide the
    next LDWEIGHTS (~233 PE cycles) under its column stream."""
    n = -(-total // 512)
    base = total // n
    rem = total - base * n
    out, o = [], 0
    for i in range(n):
        c = base + (1 if i < rem else 0)
        out.append((o, c))
        o += c
    return out


def _build_program(C):
    import concourse.tile as tile
    import concourse.mybir as mybir
    from concourse import bacc

    dt = mybir.dt
    # Bacc (not raw Bass): its compile() pass moves matmul waits onto
    # ldweights and splits over-budget sync waits into event semaphores —
    # without it walrus rejects instructions with >1 wait.
    nc = bacc.Bacc("TRN2", target_bir_lowering=False)

    xsT = nc.declare_dram_parameter("xsT", [128, KD, S], dt.float16,
                                    isOutput=False)
    suT = nc.declare_dram_parameter("suT", [128, MH, KD, 128], dt.float16,
                                    isOutput=False)
    xrT = nc.declare_dram_parameter("xrT", [128, KD, C], dt.bfloat16,
                                    isOutput=False)
    wuT = nc.declare_dram_parameter("wuT", [128, MH, KD, 128], dt.bfloat16,
                                    isOutput=False)
    sdT = nc.declare_dram_parameter("sdT", [128, MD, MH, 128], dt.float16,
                                    isOutput=False)
    wdT = nc.declare_dram_parameter("wdT", [128, MD, MH, 128], dt.bfloat16,
                                    isOutput=False)
    yT = nc.declare_dram_parameter("yT", [128, MD, C + 2 * S], dt.bfloat16,
                                   isOutput=True)

    CCH = _chunks(C)   # routed token chunks
    SCH = _chunks(S)   # shared token chunks (just one: 256)

    with ExitStack() as ctx:
        tc = ctx.enter_context(tile.TileContext(nc))
        wpool = ctx.enter_context(tc.tile_pool(name="w", bufs=1))
        hpool = ctx.enter_context(tc.tile_pool(name="h", bufs=1))
        opool = ctx.enter_context(tc.tile_pool(name="o", bufs=1))
        psR = [ctx.enter_context(
                   tc.tile_pool(name=f"psR{i}", bufs=2, space="PSUM"))
               for i in range(len(CCH))]
        psS = ctx.enter_context(tc.tile_pool(name="psS", bufs=2, space="PSUM"))

        xs = wpool.tile([128, KD, S], dt.float16, tag="xs", name="xs")
        su = wpool.tile([128, MH, KD, 128], dt.float16, tag="su", name="su")
        xr = wpool.tile([128, KD, C], dt.bfloat16, tag="xr", name="xr")
        wu = wpool.tile([128, MH, KD, 128], dt.bfloat16, tag="wu", name="wu")
        sd = wpool.tile([128, MD, MH, 128], dt.float16, tag="sd", name="sd")
        wd = wpool.tile([128, MD, MH, 128], dt.bfloat16, tag="wd", name="wd")

        # DMA triggers all on the sync engine, in exact consumption order.
        # Its 16 HWDGE queues are FIFO, so issue order == delivery order
        # with no dep chaining, and a single lane keeps the whole 420 GB/s
        # on the startup-critical xs+su stream.  A second trigger engine
        # (scalar) measurably lengthens the NEFF teardown (more queue sems
        # to drain) and steals bandwidth from the first stream.
        # xs in k-halves: the first shared tile's k0-3 matmuls only need the
        # first half, shaving ~0.5 us off the time-to-first-matmul.
        nc.sync.dma_start(xs[:, 0:KD // 2], xsT[:, 0:KD // 2])
        nc.sync.dma_start(su[:, 0:1], suT[:, 0:1])
        nc.sync.dma_start(xs[:, KD // 2:], xsT[:, KD // 2:])
        for m0, m1 in ((1, 2), (2, 3), (3, 5), (5, 8), (8, 11)):
            nc.sync.dma_start(su[:, m0:m1], suT[:, m0:m1])
        # wu[m0] + first xr k-half before the rest: the first routed tile
        # only needs xr[k0:4], so it can start ~1.3 us of delivery earlier —
        # slack that absorbs per-core HBM-contention jitter.
        nc.sync.dma_start(wu[:, 0:1], wuT[:, 0:1])
        nc.sync.dma_start(xr[:, 0:KD // 2], xrT[:, 0:KD // 2])
        nc.sync.dma_start(xr[:, KD // 2:], xrT[:, KD // 2:])
        for m0, m1 in ((1, 2), (2, 3), (3, 5), (5, 7), (7, 9), (9, 11)):
            nc.sync.dma_start(wu[:, m0:m1], wuT[:, m0:m1])
        for m0, m1 in ((0, 4), (4, 8)):
            nc.sync.dma_start(wd[:, m0:m1], wdT[:, m0:m1])
        for m0, m1 in ((0, 4), (4, 8)):
            nc.sync.dma_start(sd[:, m0:m1], sdT[:, m0:m1])

        h_t = hpool.tile([128, MH, C], dt.bfloat16, tag="h", name="h")
        g_t = hpool.tile([128, MH, S], dt.float16, tag="g", name="g")

        def up_tile(which, m):
            """One up-proj m-tile: matmuls over KD, relu^2 epilogue on DVE."""
            shared = which == 's'
            chunks = SCH if shared else CCH
            w_t = su if shared else wu
            x_t = xs if shared else xr
            dst = g_t if shared else h_t
            pss = []
            for i, (co, cw) in enumerate(chunks):
                pool = psS if shared else psR[i]
                ps = pool.tile([128, cw], dt.float32, tag=pool.name,
                               name=pool.name)
                pss.append((co, cw, ps))
            for k in range(KD):
                for (co, cw, ps) in pss:
                    nc.tensor.matmul(ps[:], w_t[:, m, k, :],
                                     x_t[:, k, co:co + cw],
                                     start=(k == 0), stop=(k == KD - 1))
            # Epilogues on DVE (ACT pays a LUT-table load per op and is
            # several times slower on plain relu/copy tiles).
            for (co, cw, ps) in pss:
                v = dst[:, m, co:co + cw]
                nc.vector.tensor_relu(v, ps[:])
                nc.vector.tensor_mul(v, v, v)

        # --- up-phase: ALL shared tiles first.  Shared input is small
        # (0.26 MB per m-tile) so the PE can start ~2 us after the first
        # trigger and stays fed at 306 GB/s while the big xr+wu stream
        # queues behind; by the time the 11 shared tiles are done (~9.4 us)
        # xr and the first wu slices have landed and the routed tiles run
        # with delivery ahead of consumption the rest of the way.
        # s10 runs AFTER the routed tiles: the down-phase starts with a
        # routed tile, whose input (all routed epilogues) is then already
        # written — r10's relu^2 drains during s10's matmuls.
        for m in range(MH - 1):
            up_tile('s', m)
        for m in range(MH):
            up_tile('r', m)
        up_tile('s', MH - 1)

        ybr = opool.tile([128, MD, C], dt.bfloat16, tag="ybr", name="ybr")
        ybs = opool.tile([128, MD, S], dt.float32, tag="ybs", name="ybs")

        def down_tile(which, m):
            shared = which == 's'
            chunks = SCH if shared else CCH
            w_t = sd if shared else wd
            x_t = g_t if shared else h_t
            dst = ybs if shared else ybr
            pss = []
            for i, (co, cw) in enumerate(chunks):
                pool = psS if shared else psR[i]
                ps = pool.tile([128, cw], dt.float32, tag=pool.name,
                               name=pool.name)
                pss.append((co, cw, ps))
            for kk in range(MH):
                for (co, cw, ps) in pss:
                    nc.tensor.matmul(ps[:], w_t[:, m, kk, :],
                                     x_t[:, kk, co:co + cw],
                                     start=(kk == 0), stop=(kk == MH - 1))
            for (co, cw, ps) in pss:
                nc.vector.tensor_copy(dst[:, m, co:co + cw], ps[:])

        # --- down-phase: alternate routed/shared; stage stores so the
        # final trigger covers only the last m-tile (smallest possible
        # wait + transfer after the last copy).
        store_at = {1: 0, 3: 2, 5: 4, 6: 6, 7: 7}
        for m in range(MD):
            down_tile('r', m)
            down_tile('s', m)
            if m in store_at:
                m0 = store_at[m]
                nc.sync.dma_start(yT[:, m0:m + 1, :C], ybr[:, m0:m + 1, :])
                nc.sync.dma_start(
                    yT[:, m0:m + 1, C:].bitcast(dt.float32),
                    ybs[:, m0:m + 1, :])

    nc.compile()
    return nc


def _route(x, gate_w, expert_bias):
    """Exact numpy mirror of the reference TopKRouter + dispatch."""
    xf = x.reshape(-1, DIM).astype(np.float32)
    logits = xf @ gate_w.T.astype(np.float32)
    scores = 1.0 / (1.0 + np.exp(-logits.astype(np.float32)))
    biased = scores + expert_bias[None, :].astype(np.float32)
    # top-2, ties -> lower index (matches jax.lax.top_k)
    sel = np.argsort(-biased, axis=-1, kind="stable")[:, :TOPK]
    top_scores = np.take_along_axis(scores, sel, axis=-1)
    flat_sel = sel.reshape(-1)
    counts = np.bincount(flat_sel, minlength=E)
    order = np.argsort(flat_sel, kind="stable")
    scores_sorted = top_scores.reshape(-1)[order]
    token_ids = order // TOPK
    return xf, counts, order, token_ids, scores_sorted


def _kchunk(mat, width):
    """(n_k*128, width) row-major -> (128, n_k, width)."""
    return mat.reshape(-1, 128, width).transpose(1, 0, 2)


def _mmajor(mat, n_m):
    """(n_k*128, n_m*128) -> (128, n_m, n_k, 128) m-major weight tiles."""
    n_k = mat.shape[0] // 128
    return mat.reshape(n_k, 128, n_m, 128).transpose(1, 2, 0, 3)


def _shrink_walrus_sem_space():
    """Cap walrus's semaphore space: the NEFF epilogue clears every sem in
    [0, max-sem-num) one instruction at a time (~26ns each across engines),
    a fixed ~6.4us tail at the default 150.  Walrus's static needs are ~78
    sems (NRT + engines + sequencers + CC + SWDGE + HWDGE + IO +
    SpillReload); 96 leaves margin.  Bass-side kernel sems live at 150+
    and are cleared separately by the tile context's one-op RANGE_CLEAR."""
    import concourse.bass_utils as bu
    if getattr(bu.get_walrus_args, "_sem_capped", False):
        return
    orig = bu.get_walrus_args

    def patched(*args, **kwargs):
        return orig(*args, **kwargs) + ["--max-sem-num=96"]

    patched._sem_capped = True
    bu.get_walrus_args = patched


def kernel(x, gate_w, expert_bias, w_up, w_down, shared_w_up, shared_w_down):
    global LAST_RESULT
    from concourse.bass_utils import run_bass_kernel_spmd
    _shrink_walrus_sem_space()

    xf, counts, order, token_ids, scores_sorted = _route(x, gate_w, expert_bias)

    # capacity per expert: exact max count, rounded to even so the f32
    # bitcast of the ys output region stays element-aligned
    C = max(256, int(counts.max() + 1) & ~1)
    starts = np.zeros(E + 1, np.int64)
    np.cumsum(counts, out=starts[1:])

    # dispatch: routed_in rows grouped by expert, scaled by router score
    routed_in = (xf[token_ids] * scores_sorted[:, None]).astype(np.float32)
    routed_in = routed_in.astype(bf16)

    su_pack = np.ascontiguousarray(
        _mmajor(shared_w_up.T.astype(np.float16), MH))
    sd_pack = np.ascontiguousarray(
        _mmajor(shared_w_down.T.astype(np.float16), MD))
    in_maps = []
    for e in range(NCORES):
        seg = routed_in[starts[e]:starts[e + 1]]
        xr = np.zeros((C, DIM), bf16)
        xr[:seg.shape[0]] = seg
        xsTe = xf[e * S:(e + 1) * S].T.astype(np.float16)   # (DIM, S)
        in_maps.append({
            "xsT": np.ascontiguousarray(_kchunk(xsTe, S)),
            "suT": su_pack,
            "xrT": np.ascontiguousarray(_kchunk(xr.T, C)),
            "wuT": np.ascontiguousarray(_mmajor(w_up[e].astype(bf16).T, MH)),
            "sdT": sd_pack,
            "wdT": np.ascontiguousarray(_mmajor(w_down[e].astype(bf16).T, MD)),
        })

    if C not in _PROG_CACHE:
        _PROG_CACHE[C] = _build_program(C)
    nc = _PROG_CACHE[C]

    res = run_bass_kernel_spmd(
        nc, in_maps, list(range(NCORES)),
        trace=TRACE,
        trace_cores=TRACE_CORES,
        tmpdir=TRACE_DIR,
    )
    LAST_RESULT = res

    # --- combine (host): scatter per-expert outputs back to token order ---
    routed_sorted = np.empty((T * TOPK, DIM), np.float32)
    for e in range(NCORES):
        arr = np.asarray(res.results[e]["yT"])             # (128, MD, C+2S)
        yr = arr[:, :, :C].transpose(1, 0, 2).reshape(DIM, C).T
        routed_sorted[starts[e]:starts[e + 1]] = yr[:counts[e]].astype(np.float32)
    combined = np.empty((T * TOPK, DIM), np.float32)
    combined[order] = routed_sorted
    out = combined.reshape(T, TOPK, DIM).sum(axis=1)

    for e in range(NCORES):
        arr = np.asarray(res.results[e]["yT"])
        ys = np.ascontiguousarray(arr[:, :, C:]).view(np.float32)  # (128,MD,S)
        out[e * S:(e + 1) * S] += ys.transpose(1, 0, 2).reshape(DIM, S).T

    return out.reshape(1, T, DIM).astype(np.float32)


# revision 13
# speedup vs baseline: 1.0213x; 1.0213x over previous
"""MoE (8 experts, top-2, shared expert) Trainium2 kernel.

Strategy (expert-parallel, per sharding hint):
  - Host computes routing (sigmoid gate -> top-2 -> stable sort by expert),
    exactly mirroring the jax reference in fp32 numpy, and plays the role of
    the all-to-all: expert e's tokens (scaled by router scores, bf16,
    zero-padded to capacity C = max expert count) go to core e. The shared
    expert is data-parallel: core i gets tokens [i*256, (i+1)*256).
  - Device does the 4 GEMMs in feature-major layout (tokens on the moving
    free dim) so no on-chip transposes are needed:
        hT  = wu.T.T @ xrT  (bf16, fp32 accum)   -> relu^2 in bf16
        yrT = wd.T.T @ hT   (bf16, fp32 accum)   -> bf16 out
        gT  = su.T.T @ xsT  (fp16)               -> relu^2
        ysT = sd.T.T @ gT   (fp16)               -> fp32 out
    fp16 (10-bit mantissa) gives ~tf32 precision at half the f32 DMA bytes.
  - Host scatters per-expert outputs back to token order, sums top-2 + shared.

Schedule notes (what made it fast):
  - Weights are streamed M-MAJOR (one DRAM param per tensor, tiled
    [128, m, k, 128]) and DMA triggers are issued in exactly the PE's
    consumption order, so the first matmul only waits for xs + su[m0]
    (~0.8 MB) instead of a whole k-major pack.  HWDGE queues are FIFO per
    engine, so trigger order == delivery order with no dep chaining.
  - Two HWDGE engines: shared-expert streams + sd + ys-stores ride the sync
    queues, routed streams + wd + yr-stores ride the scalar (ACT) queues.
  - Token chunks split C into two near-halves (>=233 cols each) so every
    LDWEIGHTS (~97ns) hides under the previous matmul's column stream; the
    baseline's (512, 64) split exposed ~90 cycles per narrow chunk.
  - Up-phase interleaves shared/routed m-tiles (shared first: its input is
    small and lands first); the last up-tile is shared so the down-phase's
    first routed tile never waits on an epilogue.
  - PSUM tile slots are bank-padded (2KB/partition) by the framework, so 6
    in-flight accumulators (2x274 + 2x273 + 2x256 fp32) fit the 8 banks.

Self-contained: hardcodes shapes from the problem spec.
"""

import numpy as np
import ml_dtypes
from contextlib import ExitStack

T = 2048          # tokens (BS*SLEN)
DIM = 1024
E = 8             # experts == cores
TOPK = 2
HID = 1408
NCORES = 8
S = T // NCORES   # shared-expert tokens per core

KD = DIM // 128   # 8  k-tiles contracting over DIM
MH = HID // 128   # 11 m-tiles over hidden
MD = DIM // 128   # 8  m-tiles over model dim

TRACE = False
TRACE_CORES = None
TRACE_DIR = None
LAST_RESULT = None   # BassKernelResults of the last run (for test harness)

_PROG_CACHE = {}

bf16 = ml_dtypes.bfloat16


def _chunks(total):
    """Split total into near-equal chunks of <=512 (PSUM bank width).
    Near-equal (not 512+rest) so every chunk is wide enough to hide the
    next LDWEIGHTS (~233 PE cycles) under its column stream."""
    n = -(-total // 512)
    base = total // n
    rem = total - base * n
    out, o = [], 0
    for i in range(n):
        c = base + (1 if i < rem else 0)
        out.append((o, c))
        o += c
    return out


def _build_program(C):
    import concourse.tile as tile
    import concourse.mybir as mybir
    from concourse import bacc

    dt = mybir.dt
    # Bacc (not raw Bass): its compile() pass moves matmul waits onto
    # ldweights and splits over-budget sync waits into event semaphores —
    # without it walrus rejects instructions with >1 wait.
    nc = bacc.Bacc("TRN2", target_bir_lowering=False)

    xsT = nc.declare_dram_parameter("xsT", [128, KD, S], dt.float16,
                                    isOutput=False)
    suT = nc.declare_dram_parameter("suT", [128, MH, KD, 128], dt.float16,
                                    isOutput=False)
    xrT = nc.declare_dram_parameter("xrT", [128, KD, C], dt.bfloat16,
                                    isOutput=False)
    wuT = nc.declare_dram_parameter("wuT", [128, MH, KD, 128], dt.bfloat16,
                                    isOutput=False)
    sdT = nc.declare_dram_parameter("sdT", [128, MD, MH, 128], dt.float16,
                                    isOutput=False)
    wdT = nc.declare_dram_parameter("wdT", [128, MD, MH, 128], dt.bfloat16,
                                    isOutput=False)
    yT = nc.declare_dram_parameter("yT", [128, MD, C + 2 * S], dt.bfloat16,
                                   isOutput=True)

    CCH = _chunks(C)   # routed token chunks
    SCH = _chunks(S)   # shared token chunks (just one: 256)

    with ExitStack() as ctx:
        tc = ctx.enter_context(tile.TileContext(nc))
        wpool = ctx.enter_context(tc.tile_pool(name="w", bufs=1))
        hpool = ctx.enter_context(tc.tile_pool(name="h", bufs=1))
        opool = ctx.enter_context(tc.tile_pool(name="o", bufs=1))
        psR = [ctx.enter_context(
                   tc.tile_pool(name=f"psR{i}", bufs=2, space="PSUM"))
               for i in range(len(CCH))]
        psS = ctx.enter_context(tc.tile_pool(name="psS", bufs=2, space="PSUM"))

        xs = wpool.tile([128, KD, S], dt.float16, tag="xs", name="xs")
        su = wpool.tile([128, MH, KD, 128], dt.float16, tag="su", name="su")
        xr = wpool.tile([128, KD, C], dt.bfloat16, tag="xr", name="xr")
        wu = wpool.tile([128, MH, KD, 128], dt.bfloat16, tag="wu", name="wu")
        sd = wpool.tile([128, MD, MH, 128], dt.float16, tag="sd", name="sd")
        wd = wpool.tile([128, MD, MH, 128], dt.bfloat16, tag="wd", name="wd")

        # DMA triggers all on the sync engine, in exact consumption order.
        # Its 16 HWDGE queues are FIFO, so issue order == delivery order
        # with no dep chaining, and a single lane keeps the whole 420 GB/s
        # on the startup-critical xs+su stream.  A second trigger engine
        # (scalar) measurably lengthens the NEFF teardown (more queue sems
        # to drain) and steals bandwidth from the first stream.
        # xs in k-halves: the first shared tile's k0-3 matmuls only need the
        # first half, shaving ~0.5 us off the time-to-first-matmul.
        nc.sync.dma_start(xs[:, 0:KD // 2], xsT[:, 0:KD // 2])
        nc.sync.dma_start(su[:, 0:1], suT[:, 0:1])
        nc.sync.dma_start(xs[:, KD // 2:], xsT[:, KD // 2:])
        for m0, m1 in ((1, 2), (2, 3), (3, 5), (5, 8), (8, 11)):
            nc.sync.dma_start(su[:, m0:m1], suT[:, m0:m1])
        # wu[m0] + first xr k-half before the rest: the first routed tile
        # only needs xr[k0:4], so it can start ~1.3 us of delivery earlier —
        # slack that absorbs per-core HBM-contention jitter.
        nc.sync.dma_start(wu[:, 0:1], wuT[:, 0:1])
        nc.sync.dma_start(xr[:, 0:KD // 2], xrT[:, 0:KD // 2])
        nc.sync.dma_start(xr[:, KD // 2:], xrT[:, KD // 2:])
        for m0, m1 in ((1, 2), (2, 3), (3, 5), (5, 7), (7, 9), (9, 11)):
            nc.sync.dma_start(wu[:, m0:m1], wuT[:, m0:m1])
        for m0, m1 in ((0, 4), (4, 8)):
            nc.sync.dma_start(wd[:, m0:m1], wdT[:, m0:m1])
        for m0, m1 in ((0, 4), (4, 8)):
            nc.sync.dma_start(sd[:, m0:m1], sdT[:, m0:m1])

        h_t = hpool.tile([128, MH, C], dt.bfloat16, tag="h", name="h")
        g_t = hpool.tile([128, MH, S], dt.float16, tag="g", name="g")

        def up_tile(which, m):
            """One up-proj m-tile: matmuls over KD, relu^2 epilogue on DVE."""
            shared = which == 's'
            chunks = SCH if shared else CCH
            w_t = su if shared else wu
            x_t = xs if shared else xr
            dst = g_t if shared else h_t
            pss = []
            for i, (co, cw) in enumerate(chunks):
                pool = psS if shared else psR[i]
                ps = pool.tile([128, cw], dt.float32, tag=pool.name,
                               name=pool.name)
                pss.append((co, cw, ps))
            for k in range(KD):
                for (co, cw, ps) in pss:
                    nc.tensor.matmul(ps[:], w_t[:, m, k, :],
                                     x_t[:, k, co:co + cw],
                                     start=(k == 0), stop=(k == KD - 1))
            # Epilogues on DVE (ACT pays a LUT-table load per op and is
            # several times slower on plain relu/copy tiles).
            for (co, cw, ps) in pss:
                v = dst[:, m, co:co + cw]
                nc.vector.tensor_relu(v, ps[:])
                nc.vector.tensor_mul(v, v, v)

        # --- up-phase: ALL shared tiles first.  Shared input is small
        # (0.26 MB per m-tile) so the PE can start ~2 us after the first
        # trigger and stays fed at 306 GB/s while the big xr+wu stream
        # queues behind; by the time the 11 shared tiles are done (~9.4 us)
        # xr and the first wu slices have landed and the routed tiles run
        # with delivery ahead of consumption the rest of the way.
        # s10 runs AFTER the routed tiles: the down-phase starts with a
        # routed tile, whose input (all routed epilogues) is then already
        # written — r10's relu^2 drains during s10's matmuls.
        for m in range(MH - 1):
            up_tile('s', m)
        for m in range(MH):
            up_tile('r', m)
        up_tile('s', MH - 1)

        ybr = opool.tile([128, MD, C], dt.bfloat16, tag="ybr", name="ybr")
        ybs = opool.tile([128, MD, S], dt.float32, tag="ybs", name="ybs")

        def down_tile(which, m):
            shared = which == 's'
            chunks = SCH if shared else CCH
            w_t = sd if shared else wd
            x_t = g_t if shared else h_t
            dst = ybs if shared else ybr
            pss = []
            for i, (co, cw) in enumerate(chunks):
                pool = psS if shared else psR[i]
                ps = pool.tile([128, cw], dt.float32, tag=pool.name,
                               name=pool.name)
                pss.append((co, cw, ps))
            for kk in range(MH):
                for (co, cw, ps) in pss:
                    nc.tensor.matmul(ps[:], w_t[:, m, kk, :],
                                     x_t[:, kk, co:co + cw],
                                     start=(kk == 0), stop=(kk == MH - 1))
            for (co, cw, ps) in pss:
                nc.vector.tensor_copy(dst[:, m, co:co + cw], ps[:])

        # --- down-phase: alternate routed/shared; stage stores so the
        # final trigger covers only the last m-tile (smallest possible
        # wait + transfer after the last copy).
        store_at = {1: 0, 3: 2, 5: 4, 6: 6, 7: 7}
        for m in range(MD):
            down_tile('r', m)
            down_tile('s', m)
            if m in store_at:
                m0 = store_at[m]
                nc.sync.dma_start(yT[:, m0:m + 1, :C], ybr[:, m0:m + 1, :])
                nc.sync.dma_start(
                    yT[:, m0:m + 1, C:].bitcast(dt.float32),
                    ybs[:, m0:m + 1, :])

    nc.compile()
    return nc


def _route(x, gate_w, expert_bias):
    """Exact numpy mirror of the reference TopKRouter + dispatch."""
    xf = x.reshape(-1, DIM).astype(np.float32)
    logits = xf @ gate_w.T.astype(np.float32)
    scores = 1.0 / (1.0 + np.exp(-logits.astype(np.float32)))
    biased = scores + expert_bias[None, :].astype(np.float32)
    # top-2, ties -> lower index (matches jax.lax.top_k)
    sel = np.argsort(-biased, axis=-1, kind="stable")[:, :TOPK]
    top_scores = np.take_along_axis(scores, sel, axis=-1)
    flat_sel = sel.reshape(-1)
    counts = np.bincount(flat_sel, minlength=E)
    order = np.argsort(flat_sel, kind="stable")
    scores_sorted = top_scores.reshape(-1)[order]
    token_ids = order // TOPK
    return xf, counts, order, token_ids, scores_sorted


def _kchunk(mat, width):
    """(n_k*128, width) row-major -> (128, n_k, width)."""
    return mat.reshape(-1, 128, width).transpose(1, 0, 2)


def _mmajor(mat, n_m):
    """(n_k*128, n_m*128) -> (128, n_m, n_k, 128) m-major weight tiles."""
    n_k = mat.shape[0] // 128
    return mat.reshape(n_k, 128, n_m, 128).transpose(1, 2, 0, 3)


def kernel(x, gate_w, expert_bias, w_up, w_down, shared_w_up, shared_w_down):
    global LAST_RESULT
    from concourse.bass_utils import run_bass_kernel_spmd

    xf, counts, order, token_ids, scores_sorted = _route(x, gate_w, expert_bias)

    # capacity per expert: exact max count, rounded to even so the f32
    # bitcast of the ys output region stays element-aligned
    C = max(256, int(counts.max() + 1) & ~1)
    starts = np.zeros(E + 1, np.int64)
    np.cumsum(counts, out=starts[1:])

    # dispatch: routed_in rows grouped by expert, scaled by router score
    routed_in = (xf[token_ids] * scores_sorted[:, None]).astype(np.float32)
    routed_in = routed_in.astype(bf16)

    su_pack = np.ascontiguousarray(
        _mmajor(shared_w_up.T.astype(np.float16), MH))
    sd_pack = np.ascontiguousarray(
        _mmajor(shared_w_down.T.astype(np.float16), MD))
    in_maps = []
    for e in range(NCORES):
        seg = routed_in[starts[e]:starts[e + 1]]
        xr = np.zeros((C, DIM), bf16)
        xr[:seg.shape[0]] = seg
        xsTe = xf[e * S:(e + 1) * S].T.astype(np.float16)   # (DIM, S)
        in_maps.append({
            "xsT": np.ascontiguousarray(_kchunk(xsTe, S)),
            "suT": su_pack,
            "xrT": np.ascontiguousarray(_kchunk(xr.T, C)),
            "wuT": np.ascontiguousarray(_mmajor(w_up[e].astype(bf16).T, MH)),
            "sdT": sd_pack,
            "wdT": np.ascontiguousarray(_mmajor(w_down[e].astype(bf16).T, MD)),
        })

    if C not in _PROG_CACHE:
        _PROG_CACHE[C] = _build_program(C)
    nc = _PROG_CACHE[C]

    res = run_bass_kernel_spmd(
        nc, in_maps, list(range(NCORES)),
        trace=TRACE,
        trace_cores=TRACE_CORES,
        tmpdir=TRACE_DIR,
    )
    LAST_RESULT = res

    # --- combine (host): scatter per-expert outputs back to token order ---
    routed_sorted = np.empty((T * TOPK, DIM), np.float32)
    for e in range(NCORES):
        arr = np.asarray(res.results[e]["yT"])             # (128, MD, C+2S)
        yr = arr[:, :, :C].transpose(1, 0, 2).reshape(DIM, C).T
        routed_sorted[starts[e]:starts[e + 1]] = yr[:counts[e]].astype(np.float32)
    combined = np.empty((T * TOPK, DIM), np.float32)
    combined[order] = routed_sorted
    out = combined.reshape(T, TOPK, DIM).sum(axis=1)

    for e in range(NCORES):
        arr = np.asarray(res.results[e]["yT"])
        ys = np.ascontiguousarray(arr[:, :, C:]).view(np.float32)  # (128,MD,S)
        out[e * S:(e + 1) * S] += ys.transpose(1, 0, 2).reshape(DIM, S).T

    return out.reshape(1, T, DIM).astype(np.float32)
